# revision 1
# baseline (speedup 1.0000x reference)
"""Trainium2 (Bass/Tile) 8-core kernel for a dense transformer block.

Math (mirrors the reference):
    q      = x @ wi                       # all heads share wi -> q == k == v
    P      = softmax(mask(q q^T / 32))
    head   = q + P @ q
    h      = head @ W_eff + bias          # cat of identical heads @ out_kernel
                                          # == head @ (sum of the 8 blocks)
    hn     = layernorm(h)                 # E[x^2]-E[x]^2 variance, eps=1e-5
    out    = silu(hn @ wi) @ wi

Sharding (8 NeuronCores, one SPMD NEFF):
    core c -> batch c//4, two 256-row strips {j, 7-j} (j = c%4) of that batch
    (balances causal attention load).  q^T and q (both layouts, bf16) are
    AllGathered within each 4-core batch group; W_eff is computed by an 8-core
    AllReduce of per-core out_kernel blocks.  The program is identical on all
    cores: every per-core difference (which rows, which weight block, the
    attention mask) is carried by input data, incl. a host-built additive
    mask tile so arbitrary masks are supported.
"""

import sys

for _p in ("/opt/trn_rl_repo",):
    if _p not in sys.path:
        sys.path.insert(0, _p)

import numpy as np

B, S, D, H = 2, 2048, 1024, 8
NCORES = 8
TOK = 512          # tokens (q rows) per core
NSTR = 8           # 256-row strips per batch
STR = 256          # strip size
KT = S // 128      # 16 k-tiles of 128
EPS = 1e-5
MASK_NEG = -1.0e6  # pre-scale additive mask value (exp(-1e6/32) == 0)

_CACHE = {}


def _strips(j):
    return (j, NSTR - 1 - j)


def _build(debug=False, reps=1, sim_cc_as_dma=False):
    import concourse.bacc as bacc
    import concourse.mybir as mybir
    import concourse.tile as tile
    from concourse.replica_groups import maybe_share_collective_output_space

    dt = mybir.dt
    BF, F32 = dt.bfloat16, dt.float32
    AF = mybir.ActivationFunctionType
    AX = mybir.AxisListType
    ALU = mybir.AluOpType

    nc = bacc.Bacc("TRN2", target_bir_lowering=False, debug=False,
                   num_devices=NCORES)

    # ---------------- I/O (per-core shapes) ----------------
    xt_d = nc.dram_tensor("xt", [D, TOK], F32, kind="ExternalInput")
    wi_d = nc.dram_tensor("wi", [D, D], F32, kind="ExternalInput")
    wo_d = nc.dram_tensor("wo", [D, D], F32, kind="ExternalInput")
    bias_d = nc.dram_tensor("bias", [1, D], F32, kind="ExternalInput")
    amask_d = nc.dram_tensor("amask", [4, 128, S], BF, kind="ExternalInput")
    out_d = nc.dram_tensor("out", [TOK, D], F32, kind="ExternalOutput")
    if debug:
        dbg = {
            "dbg_qT_my": nc.dram_tensor("dbg_qT_my", [128, 8, TOK], BF, kind="ExternalOutput"),
            "dbg_qT_all": nc.dram_tensor("dbg_qT_all", [128, 8, S], BF, kind="ExternalOutput"),
            "dbg_qn_all": nc.dram_tensor("dbg_qn_all", [128, KT, D], BF, kind="ExternalOutput"),
            "dbg_weff": nc.dram_tensor("dbg_weff", [128, 8, D], BF, kind="ExternalOutput"),
            "dbg_E": nc.dram_tensor("dbg_E", [4, 128, S], BF, kind="ExternalOutput"),
            "dbg_hT": nc.dram_tensor("dbg_hT", [2, 128, 8, STR], BF, kind="ExternalOutput"),
            "dbg_hn": nc.dram_tensor("dbg_hn", [128, 4, D], BF, kind="ExternalOutput"),
        }

    # ---------------- collective buffers -------------------
    AR_G = [list(range(NCORES))]
    AG_G = [[0, 1, 2, 3], [4, 5, 6, 7]]
    wred_in = nc.dram_tensor("wred_in", [D, D], BF)
    wred_out = nc.dram_tensor(
        "wred_out", [D, D], BF,
        addr_space=maybe_share_collective_output_space("AllReduce", AR_G))
    qtg_in = nc.dram_tensor("qtg_in", [D * TOK], BF)       # q^T pack, flat
    qtg_out = nc.dram_tensor(
        "qtg_out", [4 * D * TOK], BF,
        addr_space=maybe_share_collective_output_space("AllGather", AG_G))

    with tile.TileContext(nc) as tc:
        with (
            tc.tile_pool(name="persist", bufs=1) as pp,
            tc.tile_pool(name="load", bufs=4) as loadp,
            tc.tile_pool(name="ps", bufs=6, space="PSUM") as psp,
            tc.tile_pool(name="pv", bufs=2, space="PSUM") as pvp,
            tc.tile_pool(name="E", bufs=3) as ep,
            tc.tile_pool(name="ET", bufs=2) as etp,
            tc.tile_pool(name="hT", bufs=2) as htp,
            tc.tile_pool(name="mk", bufs=2) as mkp,
            tc.tile_pool(name="sq", bufs=1) as sqp,
            tc.tile_pool(name="outb", bufs=3) as outp,
            tc.tile_pool(name="small", bufs=1) as smp,
        ):
            # persistent SBUF tensors
            wi_bf = pp.tile([128, 8, D], BF, tag="wi_bf")
            weff_bf = pp.tile([128, 8, D], BF, tag="weff_bf")
            xt_bf = pp.tile([128, 8, TOK], BF, tag="xt_bf")
            qT_my = pp.tile([128, 8, TOK], BF, tag="qT_my")
            qT_all = pp.tile([128, 8, S], BF, tag="qT_all")
            qn_all = pp.tile([128, KT, D], BF, tag="qn_all")
            hn_sb = pp.tile([128, 4, D], BF, tag="hn_sb")
            saT = pp.tile([128, 8, TOK], BF, tag="saT")

            ones1 = smp.tile([1, 128], BF, tag="ones1")
            bias_bf = smp.tile([1, D], BF, tag="bias_bf")
            acc = smp.tile([128, 16], F32, tag="acc")
            eps_ap = smp.tile([128, 1], F32, tag="eps_ap")
            nc.vector.memset(eps_ap[:], EPS)
            rinv = smp.tile([128, 4], F32, tag="rinv")
            st = smp.tile([128, 16], F32, tag="st")

            nc.vector.memset(ones1[:], 1.0)

            for rep in range(reps):
                # ---------- phase 0: loads + casts (x, wi first: they gate q);
                # the W_eff chain (wo -> bf16 -> AllReduce) follows and overlaps
                # everything up to the out-projection.
                for i in range(4):
                    xf = loadp.tile([128, 2, TOK], F32, tag="ld")
                    nc.sync.dma_start(
                        xf[:], xt_d[256 * i:256 * (i + 1), :]
                        .rearrange("(g p) t -> p g t", p=128))
                    nc.vector.tensor_copy(xt_bf[:, 2 * i:2 * (i + 1), :], xf[:])
                for kt in range(8):
                    wf = loadp.tile([128, D], F32, tag="ld")
                    nc.sync.dma_start(wf[:], wi_d[128 * kt:128 * (kt + 1), :])
                    eng = nc.scalar if kt % 2 == 0 else nc.vector
                    if eng is nc.scalar:
                        eng.copy(wi_bf[:, kt, :], wf[:])
                    else:
                        eng.tensor_copy(wi_bf[:, kt, :], wf[:])

                # ---------- phase 1: q = x @ wi (once); q^T via DMA-transpose;
                # AllGather both layouts (bf16) within the 4-core batch group.
                qn_my = pp.tile([128, 4, D], BF, tag="hnT", name=f"qn_my{rep}")
                for tt in range(4):
                    for hhalf in range(2):
                        qn_ps = psp.tile([128, TOK], F32, tag="ps")
                        for kd in range(8):
                            nc.tensor.matmul(
                                qn_ps[:], xt_bf[:, kd, 128 * tt:128 * (tt + 1)],
                                wi_bf[:, kd, 512 * hhalf:512 * (hhalf + 1)],
                                start=(kd == 0), stop=(kd == 7))
                        nc.scalar.copy(qn_my[:, tt, 512 * hhalf:512 * (hhalf + 1)],
                                       qn_ps[:])
                for tt in range(4):
                    nc.scalar.dma_start_transpose(
                        qT_my[:, :, 128 * tt:128 * (tt + 1)], qn_my[:, tt, :])
                nc.sync.dma_start(
                    qtg_in.ap().rearrange("(m p t) -> p m t", p=128, m=8),
                    qT_my[:])
                if sim_cc_as_dma:
                    for r in range(4):
                        nc.sync.dma_start(
                            qtg_out[r * D * TOK:(r + 1) * D * TOK], qtg_in[:])
                else:
                    nc.gpsimd.collective_compute(
                        "AllGather", ALU.bypass, replica_groups=AG_G,
                        ins=[qtg_in.ap().opt()], outs=[qtg_out.ap().opt()])

                # ---------- W_eff chain (big slack: needed only at out-proj) ----
                for kt in range(8):
                    wof = loadp.tile([128, D], F32, tag="ld")
                    nc.sync.dma_start(wof[:], wo_d[128 * kt:128 * (kt + 1), :])
                    wob = loadp.tile([128, D], BF, tag="ld")
                    eng = nc.scalar if kt % 2 == 0 else nc.vector
                    if eng is nc.scalar:
                        eng.copy(wob[:], wof[:])
                    else:
                        eng.tensor_copy(wob[:], wof[:])
                    nc.sync.dma_start(wred_in[128 * kt:128 * (kt + 1), :], wob[:])
                if sim_cc_as_dma:
                    nc.sync.dma_start(wred_out[:], wred_in[:])
                else:
                    nc.gpsimd.collective_compute(
                        "AllReduce", ALU.add, replica_groups=AR_G,
                        ins=[wred_in.ap().opt()], outs=[wred_out.ap().opt()])
                for hh in range(2):
                    nc.sync.dma_start(
                        weff_bf[:, :, 512 * hh:512 * (hh + 1)],
                        wred_out.ap().rearrange("(kt p) d -> p kt d", p=128)
                        [:, :, 512 * hh:512 * (hh + 1)])

                bias_f = loadp.tile([1, D], F32, tag="ld")
                nc.sync.dma_start(bias_f[:1, :], bias_d[:1, :])
                nc.scalar.copy(bias_bf[:1, :], bias_f[:1, :])



                # ---------- phase 3: load gathered q into SBUF ----------
                # k axis is RANK-MAJOR: rank r's 512 tokens (strips r, 7-r in
                # its local order) occupy k block [512r, 512(r+1)).  The host
                # builds amask in the same permuted k order.
                for r in range(4):
                    nc.sync.dma_start(
                        qT_all[:, :, 512 * r:512 * (r + 1)],
                        qtg_out[r * D * TOK:(r + 1) * D * TOK]
                        .rearrange("(m p t) -> p m t", p=128, m=8))
                # derive q-natural (k-tile major) locally from gathered q^T:
                # one whole-row DMA transpose per d-chunk
                for dch in range(8):
                    nc.scalar.dma_start_transpose(
                        qn_all[:, :, 128 * dch:128 * (dch + 1)],
                        qT_all[:, dch, :])

                # ---------- phase 4+5: attention, out-proj, LN ----------
                # Emission is software-pipelined so each engine's in-order
                # stream never makes PE wait on a later q-tile's softmax:
                #   PE:  sc0 sc1 sc2 PV(s0) op(s0) sc3 PV(s1) op(s1)
                #   DVE: masks0/1, norm0/1, masks2, hT-adds(s0), LN(s0), ...
                E_tiles = {}
                ET_tiles = {}
                for si in range(2):
                    ET_tiles[si] = etp.tile([128, KT, STR], BF, tag="ET",
                                            name=f"ET{si}_{rep}")
                hT_tiles = {}

                def emit_scores(qt):
                    E = ep.tile([128, S], BF, tag="E", name=f"E{qt}_{rep}")
                    E_tiles[qt] = E
                    mk = mkp.tile([128, S], BF, tag="mk", name=f"mk{qt}_{rep}")
                    nc.sync.dma_start(mk[:], amask_d[qt, :, :])
                    for n in range(4):
                        sc = psp.tile([128, 512], F32, tag="ps",
                                      name=f"sc{qt}_{n}_{rep}")
                        for kd in range(8):
                            nc.tensor.matmul(
                                sc[:], qT_my[:, kd, 128 * qt:128 * (qt + 1)],
                                qT_all[:, kd, 512 * n:512 * (n + 1)],
                                start=(kd == 0), stop=(kd == 7))
                        nc.vector.tensor_add(sc[:], sc[:],
                                             mk[:, 512 * n:512 * (n + 1)])
                        nc.scalar.activation(
                            E[:, 512 * n:512 * (n + 1)], sc[:], AF.Exp,
                            bias=0.0, scale=1.0 / 32.0,
                            accum_out=acc[:, 4 * qt + n:4 * qt + n + 1])

                def emit_norm(qt):
                    E = E_tiles[qt]
                    nc.vector.reduce_sum(rinv[:, qt:qt + 1],
                                         acc[:, 4 * qt:4 * qt + 4], axis=AX.X)
                    nc.vector.reciprocal(rinv[:, qt:qt + 1], rinv[:, qt:qt + 1])
                    nc.vector.tensor_scalar_mul(E[:], E[:], rinv[:, qt:qt + 1])
                    if debug:
                        nc.sync.dma_start(dbg["dbg_E"][qt], E[:])
                    si, tl = divmod(qt, 2)
                    nc.scalar.dma_start_transpose(
                        ET_tiles[si][:, :, 128 * tl:128 * (tl + 1)], E[:, :])

                def emit_pv(si):
                    ET = ET_tiles[si]
                    hT = htp.tile([128, 8, STR], BF, tag="hT",
                                  name=f"hT{si}_{rep}")
                    hT_tiles[si] = hT
                    for m in range(8):
                        pv = pvp.tile([128, STR], F32, tag="pv",
                                      name=f"pv{si}_{m}_{rep}")
                        for kt in range(KT):
                            nc.tensor.matmul(
                                pv[:], qn_all[:, kt, 128 * m:128 * (m + 1)],
                                ET[:, kt, :], start=(kt == 0),
                                stop=(kt == KT - 1))
                        nc.vector.tensor_add(
                            hT[:, m, :], pv[:],
                            qT_my[:, m, STR * si:STR * (si + 1)])
                        if debug:
                            nc.sync.dma_start(dbg["dbg_hT"][si, :, m, :],
                                              hT[:, m, :])

                def emit_outproj(si):
                    hT = hT_tiles[si]
                    for tl2 in range(2):
                        qt2 = 2 * si + tl2
                        hps = []
                        for hh in range(2):
                            hp = psp.tile([128, 512], F32, tag="ps",
                                          name=f"hp{qt2}_{hh}_{rep}")
                            for kd in range(8):
                                nc.tensor.matmul(
                                    hp[:], hT[:, kd, 128 * tl2:128 * (tl2 + 1)],
                                    weff_bf[:, kd, 512 * hh:512 * (hh + 1)],
                                    start=(kd == 0), stop=False)
                            nc.tensor.matmul(
                                hp[:], ones1[:1, :],
                                bias_bf[:1, 512 * hh:512 * (hh + 1)],
                                start=False, stop=True)
                            hps.append(hp)
                        # LN: mean/var from sums + sums of squares
                        c0 = 4 * qt2
                        for hh, hp in enumerate(hps):
                            nc.vector.reduce_sum(st[:, c0 + hh:c0 + hh + 1],
                                                 hp[:], axis=AX.X)
                            sqs = sqp.tile([128, 512], F32, tag="sq",
                                           name=f"sq{qt2}_{hh}_{rep}")
                            nc.scalar.activation(
                                sqs[:], hp[:], AF.Square,
                                accum_out=st[:, c0 + 2 + hh:c0 + 3 + hh])
                        mean = smp.tile([128, 4], F32, tag=f"mean{qt2}",
                                        name=f"mean{qt2}_{rep}")
                        nc.vector.tensor_scalar(
                            mean[:, 0:1], st[:, c0:c0 + 1],
                            st[:, c0 + 1:c0 + 2], 1.0 / D,
                            op0=ALU.add, op1=ALU.mult)
                        nc.vector.tensor_scalar(
                            mean[:, 1:2], st[:, c0 + 2:c0 + 3],
                            st[:, c0 + 3:c0 + 4], 1.0 / D,
                            op0=ALU.add, op1=ALU.mult)
                        nc.vector.tensor_tensor(
                            mean[:, 2:3], mean[:, 0:1], mean[:, 0:1],
                            op=ALU.mult)
                        nc.vector.tensor_tensor(
                            mean[:, 2:3], mean[:, 1:2], mean[:, 2:3],
                            op=ALU.subtract)
                        nc.scalar.activation(mean[:, 2:3], mean[:, 2:3],
                                             AF.Sqrt, bias=eps_ap[:, 0:1])
                        nc.vector.reciprocal(mean[:, 2:3], mean[:, 2:3])
                        nc.vector.tensor_scalar(
                            mean[:, 3:4], mean[:, 0:1], mean[:, 2:3], -1.0,
                            op0=ALU.mult, op1=ALU.mult)
                        for hh, hp in enumerate(hps):
                            nc.vector.tensor_scalar(
                                hn_sb[:, qt2, 512 * hh:512 * (hh + 1)], hp[:],
                                mean[:, 2:3], mean[:, 3:4],
                                op0=ALU.mult, op1=ALU.add)

                emit_scores(0)
                emit_scores(1)
                emit_norm(0)
                emit_norm(1)
                emit_scores(2)
                emit_pv(0)
                emit_norm(2)
                emit_scores(3)
                emit_norm(3)
                emit_outproj(0)
                emit_pv(1)
                emit_outproj(1)

                if debug:
                    nc.sync.dma_start(dbg["dbg_qT_my"][:], qT_my[:])
                    nc.sync.dma_start(dbg["dbg_qT_all"][:], qT_all[:])
                    nc.sync.dma_start(dbg["dbg_qn_all"][:], qn_all[:])
                    nc.sync.dma_start(dbg["dbg_weff"][:], weff_bf[:])
                    nc.sync.dma_start(dbg["dbg_hn"][:], hn_sb[:])

                # ---------- phase 6: FFN (token-halves pipelined vs LN) ----------
                hnT = pp.tile([128, 8, TOK], BF, tag="hnT", name=f"hnT{rep}")
                for tt in range(4):
                    nc.scalar.dma_start_transpose(
                        hnT[:, :, 128 * tt:128 * (tt + 1)], hn_sb[:, tt, :])
                for th in range(2):              # token half = strip
                    for m in range(8):
                        f1 = psp.tile([128, STR], F32, tag="ps",
                                      name=f"f1_{rep}_{th}_{m}")
                        for kd in range(8):
                            nc.tensor.matmul(
                                f1[:], wi_bf[:, kd, 128 * m:128 * (m + 1)],
                                hnT[:, kd, STR * th:STR * (th + 1)],
                                start=(kd == 0), stop=(kd == 7))
                        nc.scalar.activation(saT[:, m, STR * th:STR * (th + 1)],
                                             f1[:], AF.Silu)
                    for tt in (2 * th, 2 * th + 1):
                        for hh in range(2):
                            f2 = psp.tile([128, 512], F32, tag="ps",
                                          name=f"f2_{rep}_{tt}_{hh}")
                            for kd in range(8):
                                nc.tensor.matmul(
                                    f2[:], saT[:, kd, 128 * tt:128 * (tt + 1)],
                                    wi_bf[:, kd, 512 * hh:512 * (hh + 1)],
                                    start=(kd == 0), stop=(kd == 7))
                            ob = outp.tile([128, 512], F32, tag="outb",
                                           name=f"ob_{rep}_{tt}_{hh}")
                            nc.scalar.copy(ob[:], f2[:])
                            nc.sync.dma_start(
                                out_d[128 * tt:128 * (tt + 1),
                                      512 * hh:512 * (hh + 1)], ob[:])

    nc.compile()
    return nc


def _get_nc(debug=False, reps=1, sim_cc_as_dma=False):
    key = ("nc", debug, reps, sim_cc_as_dma)
    if key not in _CACHE:
        _CACHE[key] = _build(debug, reps, sim_cc_as_dma)
    return _CACHE[key]


def make_in_maps(x, mask, wi, out_kernel, out_bias):
    """Host-side sharding: build the 8 per-core input dicts."""
    import ml_dtypes

    x = np.ascontiguousarray(x, dtype=np.float32)
    wi = np.ascontiguousarray(wi, dtype=np.float32)
    out_kernel = np.ascontiguousarray(out_kernel, dtype=np.float32)
    bias = np.ascontiguousarray(out_bias, dtype=np.float32).reshape(1, D)
    mask = np.asarray(mask).astype(bool)

    # additive pre-scale mask (0 keep / -1e6 drop), bf16.
    # k columns are permuted to the kernel's rank-major token order:
    # rank r's block = [strip r | strip 7-r].
    perm = np.concatenate([np.r_[STR * s:STR * (s + 1)]
                           for r in range(4) for s in _strips(r)])
    amask_full = np.where(mask, np.float32(0.0), np.float32(MASK_NEG)) \
        .astype(ml_dtypes.bfloat16)[:, perm]

    in_maps = []
    for c in range(NCORES):
        b, j = divmod(c, 4)
        s_a, s_b = _strips(j)
        rows = np.r_[STR * s_a:STR * (s_a + 1), STR * s_b:STR * (s_b + 1)]
        xt = np.ascontiguousarray(x[b, rows, :].T)          # [D, TOK]
        amask = np.ascontiguousarray(
            amask_full[rows, :].reshape(4, 128, S))
        wo = np.ascontiguousarray(out_kernel[D * c:D * (c + 1), :])
        in_maps.append({
            "xt": xt, "wi": wi, "wo": wo, "bias": bias, "amask": amask,
        })
    return in_maps


def assemble_output(results):
    out = np.empty((B, S, D), dtype=np.float32)
    for c in range(NCORES):
        b, j = divmod(c, 4)
        s_a, s_b = _strips(j)
        res = results[c]["out"]
        out[b, STR * s_a:STR * (s_a + 1), :] = res[0:STR, :]
        out[b, STR * s_b:STR * (s_b + 1), :] = res[STR:TOK, :]
    return out


def kernel(x, mask, wi, out_kernel, out_bias, n_heads):
    from concourse.bass_utils import run_bass_kernel_spmd

    assert int(np.asarray(n_heads)) == H
    nc = _get_nc()
    in_maps = make_in_maps(x, mask, wi, out_kernel, out_bias)
    res = run_bass_kernel_spmd(nc, in_maps, core_ids=list(range(NCORES)))
    return assemble_output(res.results)


if __name__ == "__main__":
    # quick self-check against the reference if available
    sys.path.insert(0, "/root/problem")
    import reference

    inputs = {k: np.asarray(v) for k, v in reference.setup_inputs().items()}
    exp = np.asarray(reference.reference(**reference.setup_inputs()))
    act = kernel(**inputs)
    err = np.linalg.norm(act - exp) / np.linalg.norm(exp)
    print("Relative error:", err)



# revision 51
# speedup vs baseline: 1.4000x; 1.4000x over previous
"""Trainium2 (Bass/Tile) 8-core kernel for a dense transformer block.

Math (mirrors the reference):
    q      = x @ wi                       # all heads share wi -> q == k == v
    P      = softmax(mask(q q^T / 32))
    head   = q + P @ q
    h      = head @ W_eff + bias          # cat of identical heads @ out_kernel
                                          # == head @ (sum of the 8 blocks)
    hn     = layernorm(h)                 # E[x^2]-E[x]^2 variance, eps=1e-5
    out    = silu(hn @ wi) @ wi

Sharding (8 NeuronCores, one SPMD NEFF), causal fast path:
    core c -> batch c//4, rank g=c%4, owning global 128-row q-tiles
    {g, 7-g, 8+g, 15-g} (anti-diagonal assignment).  With k columns kept in
    gather order (tile-pair major, rank minor), every core's local q-tile t
    needs exactly the first (t+1) 512-column score blocks -- the causal
    attention work is identical on all cores, so one SPMD program computes
    only 10/16 score blocks and 40/64 PV k-tiles.  Only the diagonal score
    block of each tile needs masking (host-built additive tile); all other
    computed blocks are fully-keep under the causal mask.

    Host pre-casts x/wi/out_kernel/bias to bf16, so no on-device casts and
    the W_eff AllReduce reads the wo input directly.  q^T is AllGathered in
    two halves (local tiles {0,1} then {2,3}) within each 4-core batch
    group so scores start as early as possible.

    A non-causal mask falls back to the original dense kernel (_build_dense).
"""

import sys

for _p in ("/opt/trn_rl_repo",):
    if _p not in sys.path:
        sys.path.insert(0, _p)

import numpy as np

B, S, D, H = 2, 2048, 1024, 8
NCORES = 8
TOK = 512          # tokens (q rows) per core
KT = S // 128      # 16 k-tiles of 128
EPS = 1e-5
MASK_NEG = -1.0e6  # pre-scale additive mask value (exp(-1e6/32) == 0)
ACC_OFF = [0, 1, 3, 6]   # acc column base per local tile (t+1 blocks each)

_CACHE = {}


def _gtiles(g):
    """Global 128-row q-tile indices owned by rank g, in local order."""
    return (g, 7 - g, 8 + g, 15 - g)


def _build(debug=False, reps=1, sim_cc_as_dma=False, coresim_safe=False):
    import concourse.bacc as bacc
    import concourse.mybir as mybir
    import concourse.tile as tile
    from concourse.replica_groups import maybe_share_collective_output_space

    dt = mybir.dt
    BF, F32 = dt.bfloat16, dt.float32
    AF = mybir.ActivationFunctionType
    AX = mybir.AxisListType
    ALU = mybir.AluOpType

    nc = bacc.Bacc("TRN2", target_bir_lowering=False, debug=False,
                   num_devices=NCORES)

    # ---------------- I/O (per-core shapes, host pre-cast to bf16) --------
    xt_d = nc.dram_tensor("xt", [D, TOK], BF, kind="ExternalInput")
    wi_d = nc.dram_tensor("wi", [D, D], BF, kind="ExternalInput")
    wo_d = nc.dram_tensor("wo", [D, D], BF, kind="ExternalInput")
    bias_d = nc.dram_tensor("bias", [1, D], BF, kind="ExternalInput")
    dmask_d = nc.dram_tensor("dmask", [4, 128, 512], BF, kind="ExternalInput")
    ident_d = nc.dram_tensor("ident", [128, 128], BF, kind="ExternalInput")
    out_d = nc.dram_tensor("out", [TOK, D], F32, kind="ExternalOutput")
    if debug == 2:   # light taps only: qn_all + qT_all
        dbg = {
            "dbg_qT_all": nc.dram_tensor("dbg_qT_all", [128, 8, S],
                                         BF, kind="ExternalOutput"),
            "dbg_qn_all": nc.dram_tensor("dbg_qn_all", [128, KT, D], BF,
                                         kind="ExternalOutput"),
        }
    elif debug:
        dbg = {
            "dbg_qT_my": nc.dram_tensor("dbg_qT_my", [128, 8, TOK], BF,
                                        kind="ExternalOutput"),
            "dbg_qT_all": nc.dram_tensor("dbg_qT_all", [128, 8, S],
                                         BF, kind="ExternalOutput"),
            "dbg_qn_all": nc.dram_tensor("dbg_qn_all", [128, KT, D], BF,
                                         kind="ExternalOutput"),
            "dbg_E": nc.dram_tensor("dbg_E", [128, 5120], BF,
                                    kind="ExternalOutput"),
            "dbg_hT": nc.dram_tensor("dbg_hT", [128, 8, TOK], BF,
                                     kind="ExternalOutput"),
            "dbg_hn": nc.dram_tensor("dbg_hn", [128, 4, D], BF,
                                     kind="ExternalOutput"),
            "dbg_weff": nc.dram_tensor("dbg_weff", [128, 8, D], BF,
                                       kind="ExternalOutput"),
        }

    # ---------------- collective buffers -------------------
    AR_G = [list(range(NCORES))]
    AG_G = [[0, 1, 2, 3], [4, 5, 6, 7]]
    HTOK = 256  # tokens per gather half
    qtg_in = [nc.dram_tensor(f"qtg_in{i}", [D * HTOK], BF) for i in range(2)]
    qtg_out = [nc.dram_tensor(
        f"qtg_out{i}", [4 * D * HTOK], BF,
        addr_space=maybe_share_collective_output_space("AllGather", AG_G))
        for i in range(2)]
    wred_in = nc.dram_tensor("wred_in", [D, D], BF)
    wred_out = nc.dram_tensor(
        "wred_out", [D, D], BF,
        addr_space=maybe_share_collective_output_space("AllReduce", AR_G))
    # tiny AllReduce fences after each gather: an AllGather's completion
    # semaphore does NOT cover peer pushes into our output buffer, but an
    # AllReduce cannot complete without every peer's contribution, and
    # per-peer write ordering makes its arrival imply the gather data landed.
    fence_in = [nc.dram_tensor(f"fence_in{i}", [1, 64], BF) for i in range(2)]
    fence_out = [nc.dram_tensor(
        f"fence_out{i}", [1, 64], BF,
        addr_space=maybe_share_collective_output_space("AllReduce", AG_G))
        for i in range(2)]

    with tile.TileContext(nc) as tc:
        with (
            tc.tile_pool(name="persist", bufs=1) as pp,
            tc.tile_pool(name="ps", bufs=4, space="PSUM") as psp,
            tc.tile_pool(name="pv", bufs=2, space="PSUM") as pvp,
            tc.tile_pool(name="tp", bufs=2, space="PSUM") as tpp,
            tc.tile_pool(name="sq", bufs=2) as sqp,
            tc.tile_pool(name="outb", bufs=3) as outp,
            tc.tile_pool(name="small", bufs=1) as smp,
        ):
            # persistent SBUF tensors
            wi_bf = pp.tile([128, 8, D], BF, tag="wi_bf")
            weff_bf = pp.tile([128, 8, D], BF, tag="weff_bf")
            xt_bf = pp.tile([128, 8, TOK], BF, tag="xt_bf")
            qT_my = pp.tile([128, 8, TOK], BF, tag="qT_my")
            # gathered q^T: [dchunk-part, m, k] with k = 1024*lp + 256*r + t
            # 5-D: [dchunk-part, m, lp, r, 256tok] -- matmul rhs and DMA dst
            # use NATIVE slices only (a .rearrange() on an SBUF tile AP loses
            # the tile back-reference and the dep tracker misses the RAW edge)
            qT_all = pp.tile([128, 8, 2, 4, HTOK], BF, tag="qT_all")
            qn_all = pp.tile([128, KT, D], BF, tag="qn_all")
            ident = pp.tile([128, 128], BF, tag="ident")
            hT = pp.tile([128, 8, TOK], BF, tag="hT")
            hn_sb = pp.tile([128, 4, D], BF, tag="hn_sb")
            saT = pp.tile([128, 8, TOK], BF, tag="saT")
            dmask_sb = pp.tile([128, 4, 512], BF, tag="dmask")

            ones1 = smp.tile([1, 128], BF, tag="ones1")
            bias_bf = smp.tile([1, D], BF, tag="bias_bf")
            acc = smp.tile([128, 10], F32, tag="acc")
            eps_ap = smp.tile([128, 1], F32, tag="eps_ap")
            rinv = smp.tile([128, 4], F32, tag="rinv")
            st = smp.tile([128, 16], F32, tag="st")
            fence_sb = smp.tile([1, 2, 64], BF, tag="fence_sb")
            zfence = smp.tile([1, 64], BF, tag="zfence")
            nc.vector.memset(eps_ap[:], EPS)
            nc.vector.memset(ones1[:], 1.0)
            nc.vector.memset(zfence[:], 0.0)
            for i in range(2):
                nc.sync.dma_start(fence_in[i][:1, :], zfence[:1, :])

            def pos(nb, r):
                """qn_all k-tile index for score-block nb, rank r."""
                return 8 * (nb // 2) + 2 * r + (nb % 2)

            for rep in range(reps):
                R = f"_{rep}"
                # E / ET tiles (per local q-tile, prefix-sized)
                E_t = [pp.tile([128, 512 * (t + 1)], BF, tag=f"E{t}",
                               name=f"E{t}{R}") for t in range(4)]
                ET_t = [pp.tile([128, 4 * (t + 1), 128], BF, tag=f"ET{t}",
                                name=f"ET{t}{R}") for t in range(4)]
                hnT = pp.tile([128, 8, TOK], BF, tag="hnT2", name=f"hnT{R}")

                # ---------- loads (no casts: everything bf16 from host) ----
                for i in range(4):
                    nc.sync.dma_start(
                        xt_bf[:, 2 * i:2 * (i + 1), :],
                        xt_d[256 * i:256 * (i + 1), :]
                        .rearrange("(g p) t -> p g t", p=128))
                for kt in range(8):
                    nc.sync.dma_start(wi_bf[:, kt, :],
                                      wi_d[128 * kt:128 * (kt + 1), :])
                for t in range(4):
                    nc.sync.dma_start(dmask_sb[:, t, :], dmask_d[t])
                nc.sync.dma_start(bias_bf[:1, :], bias_d[:1, :])
                nc.sync.dma_start(ident[:], ident_d[:])

                # ---------- q-proj, directly transposed ---------------------
                # qT[dout, tok] = sum_d wi[d, dout] * xT[d, tok]: both
                # operands are d-on-partition as stored, so the PE emits q^T
                # natively and qT_my is engine-written (no DMA transpose, no
                # DMA-write -> DMA-read race against the gather store).
                def emit_qproj(lp):
                    for blk in range(8):
                        qp = pvp.tile([128, 256], F32, tag="pv",
                                      name=f"qp{lp}_{blk}{R}")
                        for kd in range(8):
                            nc.tensor.matmul(
                                qp[:], wi_bf[:, kd, 128 * blk:128 * (blk + 1)],
                                xt_bf[:, kd, 256 * lp:256 * (lp + 1)],
                                start=(kd == 0), stop=(kd == 7))
                        nc.scalar.copy(
                            qT_my[:, blk, 256 * lp:256 * (lp + 1)], qp[:])

                def emit_gather(lp):
                    nc.sync.dma_start(
                        qtg_in[lp].ap().rearrange("(p m t) -> p m t",
                                                  p=128, m=8),
                        qT_my[:, :, HTOK * lp:HTOK * (lp + 1)])
                    if sim_cc_as_dma:
                        for r in range(4):
                            nc.sync.dma_start(
                                qtg_out[lp][r * D * HTOK:(r + 1) * D * HTOK],
                                qtg_in[lp][:])
                    else:
                        nc.gpsimd.collective_compute(
                            "AllGather", ALU.bypass, replica_groups=AG_G,
                            ins=[qtg_in[lp].ap().opt()],
                            outs=[qtg_out[lp].ap().opt()])
                    if sim_cc_as_dma:
                        nc.sync.dma_start(fence_out[lp][:], fence_in[lp][:])
                    else:
                        nc.gpsimd.collective_compute(
                            "AllReduce", ALU.add, replica_groups=AG_G,
                            ins=[fence_in[lp].ap().opt()],
                            outs=[fence_out[lp].ap().opt()])

                def emit_qload(lp):
                    # barrier: block the SP queue on the fence AllReduce so
                    # every following load sees landed peer data
                    nc.sync.dma_start(fence_sb[:1, lp, :],
                                      fence_out[lp][:1, :])
                    # 4x512KB loads: each rank's half-tokens into gather order
                    for r in range(4):
                        nc.sync.dma_start(
                            qT_all[:, :, lp, r, :],
                            qtg_out[lp][r * D * HTOK:(r + 1) * D * HTOK]
                            .rearrange("(p m t) -> p m t", p=128, m=8))

                def emit_qntrans(lp):
                    # derive q natural (k-on-partition) via PE transposes
                    # (DmaTransposeAnt's completion semaphore fires before its
                    # tail tiles are committed -- engine semantics are exact)
                    for r in range(4):
                        for lh in range(2):
                            p_ = 8 * lp + 2 * r + lh
                            for h in range(2):
                                tps = tpp.tile([128, 512], BF, tag="tp",
                                               name=f"qtp{p_}_{h}{R}")
                                for j in range(4):
                                    nc.tensor.transpose(
                                        tps[:, 128 * j:128 * (j + 1)],
                                        qT_all[:, 4 * h + j, lp, r,
                                               128 * lh:128 * (lh + 1)],
                                        ident[:])
                                eng = nc.scalar if (p_ + h) % 2 == 0 \
                                    else nc.vector
                                if eng is nc.scalar:
                                    eng.copy(qn_all[:, p_,
                                                    512 * h:512 * (h + 1)],
                                             tps[:])
                                else:
                                    eng.tensor_copy(
                                        qn_all[:, p_, 512 * h:512 * (h + 1)],
                                        tps[:])

                emit_qproj(0)
                emit_gather(0)
                emit_qproj(1)
                emit_gather(1)

                # ---------- W_eff chain: AllReduce the bf16 wo input -------
                # (collectives cannot read IO tensors: stage via one
                #  dram-to-dram copy)
                nc.sync.dma_start(wred_in[:], wo_d[:])
                if sim_cc_as_dma:
                    nc.sync.dma_start(wred_out[:], wred_in[:])
                else:
                    nc.gpsimd.collective_compute(
                        "AllReduce", ALU.add, replica_groups=AR_G,
                        ins=[wred_in.ap().opt()], outs=[wred_out.ap().opt()])

                emit_qload(0)
                emit_qload(1)
                emit_qntrans(0)

                for hh in range(2):
                    nc.sync.dma_start(
                        weff_bf[:, :, 512 * hh:512 * (hh + 1)],
                        wred_out.ap().rearrange("(kt p) d -> p kt d", p=128)
                        [:, :, 512 * hh:512 * (hh + 1)])

                # ---------- scores ------------------------------------
                def emit_score_block(t, nb):
                    sc = psp.tile([128, 512], F32, tag="ps",
                                  name=f"sc{t}_{nb}{R}")
                    # per-rank rhs slices: each read is a contiguous 128-col
                    # window inside ONE load's dst region, the only overlap
                    # shape the dep tracker reliably detects (a combined
                    # (4r x 128) strided rhs gets NO load dependency at all)
                    for r in range(4):
                        for kd in range(8):
                            nc.tensor.matmul(
                                sc[:, 128 * r:128 * (r + 1)],
                                qT_my[:, kd, 128 * t:128 * (t + 1)],
                                qT_all[:, kd, nb // 2, r,
                                       128 * (nb % 2):128 * (nb % 2) + 128],
                                start=(kd == 0), stop=(kd == 7))
                    if nb == t:   # diagonal block: apply host-built mask
                        nc.vector.tensor_add(sc[:], sc[:], dmask_sb[:, t, :])
                    nc.scalar.activation(
                        E_t[t][:, 512 * nb:512 * (nb + 1)], sc[:], AF.Exp,
                        bias=0.0, scale=1.0 / 32.0,
                        accum_out=acc[:, ACC_OFF[t] + nb:ACC_OFF[t] + nb + 1])

                def emit_norm(t):
                    off = ACC_OFF[t]
                    nc.vector.reduce_sum(rinv[:, t:t + 1],
                                         acc[:, off:off + t + 1], axis=AX.X)
                    nc.vector.reciprocal(rinv[:, t:t + 1], rinv[:, t:t + 1])
                    nc.vector.tensor_scalar_mul(E_t[t][:], E_t[t][:],
                                                rinv[:, t:t + 1])
                    for nb in range(t + 1):
                        tps = tpp.tile([128, 512], BF, tag="tp",
                                       name=f"etp{t}_{nb}{R}")
                        for r in range(4):
                            nc.tensor.transpose(
                                tps[:, 128 * r:128 * (r + 1)],
                                E_t[t][:, 512 * nb + 128 * r:
                                        512 * nb + 128 * (r + 1)],
                                ident[:])
                        eng = nc.scalar if nb % 2 == 0 else nc.vector
                        if eng is nc.scalar:
                            eng.copy(ET_t[t][:, 4 * nb:4 * (nb + 1), :],
                                     tps[:])
                        else:
                            eng.tensor_copy(
                                ET_t[t][:, 4 * nb:4 * (nb + 1), :], tps[:])

                # wave 1: everything that needs only gather half A
                emit_score_block(0, 0)
                emit_score_block(1, 0)
                emit_score_block(1, 1)
                emit_score_block(2, 0)
                emit_score_block(2, 1)
                emit_norm(0)
                emit_score_block(3, 0)
                emit_score_block(3, 1)
                emit_norm(1)

                # ---------- PV + head (hT = q + P@q, transposed) -----------
                def emit_pv(tp):
                    for m in range(8):
                        pv = pvp.tile([128, 256], F32, tag="pv",
                                      name=f"pv{tp}_{m}{R}")
                        for tl in range(2):
                            t = 2 * tp + tl
                            pairs = [(nb, r) for nb in range(t + 1)
                                     for r in range(4)]
                            for i, (nb, r) in enumerate(pairs):
                                nc.tensor.matmul(
                                    pv[:, 128 * tl:128 * (tl + 1)],
                                    qn_all[:, pos(nb, r),
                                           128 * m:128 * (m + 1)],
                                    ET_t[t][:, 4 * nb + r, :],
                                    start=(i == 0), stop=(i == len(pairs) - 1))
                        nc.vector.tensor_add(
                            hT[:, m, 256 * tp:256 * (tp + 1)], pv[:],
                            qT_my[:, m, 256 * tp:256 * (tp + 1)])

                # ---------- out-proj + LN per local tile -------------------
                def emit_outproj(t):
                    hps = []
                    for hh in range(2):
                        hp = psp.tile([128, 512], F32, tag="ps",
                                      name=f"hp{t}_{hh}{R}")
                        for kd in range(8):
                            nc.tensor.matmul(
                                hp[:], hT[:, kd, 128 * t:128 * (t + 1)],
                                weff_bf[:, kd, 512 * hh:512 * (hh + 1)],
                                start=(kd == 0), stop=False)
                        nc.tensor.matmul(
                            hp[:], ones1[:1, :],
                            bias_bf[:1, 512 * hh:512 * (hh + 1)],
                            start=False, stop=True)
                        hps.append(hp)
                    c0 = 4 * t
                    for hh, hp in enumerate(hps):
                        nc.vector.reduce_sum(st[:, c0 + hh:c0 + hh + 1],
                                             hp[:], axis=AX.X)
                        sqs = sqp.tile([128, 512], F32, tag="sq",
                                       name=f"sq{t}_{hh}{R}")
                        nc.scalar.activation(
                            sqs[:], hp[:], AF.Square,
                            accum_out=st[:, c0 + 2 + hh:c0 + 3 + hh])
                    mean = smp.tile([128, 4], F32, tag=f"mean{t}",
                                    name=f"mean{t}{R}")
                    nc.vector.tensor_scalar(
                        mean[:, 0:1], st[:, c0:c0 + 1],
                        st[:, c0 + 1:c0 + 2], 1.0 / D,
                        op0=ALU.add, op1=ALU.mult)
                    nc.vector.tensor_scalar(
                        mean[:, 1:2], st[:, c0 + 2:c0 + 3],
                        st[:, c0 + 3:c0 + 4], 1.0 / D,
                        op0=ALU.add, op1=ALU.mult)
                    nc.vector.tensor_tensor(
                        mean[:, 2:3], mean[:, 0:1], mean[:, 0:1], op=ALU.mult)
                    nc.vector.tensor_tensor(
                        mean[:, 2:3], mean[:, 1:2], mean[:, 2:3],
                        op=ALU.subtract)
                    nc.scalar.activation(mean[:, 2:3], mean[:, 2:3],
                                         AF.Sqrt, bias=eps_ap[:, 0:1])
                    nc.vector.reciprocal(mean[:, 2:3], mean[:, 2:3])
                    nc.vector.tensor_scalar(
                        mean[:, 3:4], mean[:, 0:1], mean[:, 2:3], -1.0,
                        op0=ALU.mult, op1=ALU.mult)
                    for hh, hp in enumerate(hps):
                        nc.vector.tensor_scalar(
                            hn_sb[:, t, 512 * hh:512 * (hh + 1)], hp[:],
                            mean[:, 2:3], mean[:, 3:4],
                            op0=ALU.mult, op1=ALU.add)
                    for h in range(2):
                        tps = tpp.tile([128, 512], BF, tag="tp",
                                       name=f"htp{t}_{h}{R}")
                        for j in range(4):
                            dch = 4 * h + j
                            nc.tensor.transpose(
                                tps[:, 128 * j:128 * (j + 1)],
                                hn_sb[:, t, 128 * dch:128 * (dch + 1)],
                                ident[:])
                        for j in range(4):
                            dch = 4 * h + j
                            eng = nc.scalar if j % 2 == 0 else nc.vector
                            if eng is nc.scalar:
                                eng.copy(hnT[:, dch, 128 * t:128 * (t + 1)],
                                         tps[:, 128 * j:128 * (j + 1)])
                            else:
                                eng.tensor_copy(
                                    hnT[:, dch, 128 * t:128 * (t + 1)],
                                    tps[:, 128 * j:128 * (j + 1)])

                # ---------- FFN per token half -----------------------------
                def emit_ffn(tp):
                    for m in range(8):
                        f1 = pvp.tile([128, 256], F32, tag="pv",
                                      name=f"f1_{tp}_{m}{R}")
                        for kd in range(8):
                            nc.tensor.matmul(
                                f1[:], wi_bf[:, kd, 128 * m:128 * (m + 1)],
                                hnT[:, kd, 256 * tp:256 * (tp + 1)],
                                start=(kd == 0), stop=(kd == 7))
                        nc.scalar.activation(
                            saT[:, m, 256 * tp:256 * (tp + 1)], f1[:],
                            AF.Sigmoid if coresim_safe else AF.Silu)
                    for tl in range(2):
                        t = 2 * tp + tl
                        for hh in range(2):
                            f2 = psp.tile([128, 512], F32, tag="ps",
                                          name=f"f2_{t}_{hh}{R}")
                            for kd in range(8):
                                nc.tensor.matmul(
                                    f2[:], saT[:, kd, 128 * t:128 * (t + 1)],
                                    wi_bf[:, kd, 512 * hh:512 * (hh + 1)],
                                    start=(kd == 0), stop=(kd == 7))
                            ob = outp.tile([128, 512], F32, tag="outb",
                                           name=f"ob{t}_{hh}{R}")
                            nc.scalar.copy(ob[:], f2[:])
                            nc.sync.dma_start(
                                out_d[128 * t:128 * (t + 1),
                                      512 * hh:512 * (hh + 1)], ob[:])

                emit_pv(0)
                # wave 2: blocks needing gather half B
                emit_score_block(2, 2)
                emit_norm(2)
                emit_score_block(3, 2)
                emit_score_block(3, 3)
                emit_qntrans(1)
                emit_norm(3)
                emit_outproj(0)
                emit_outproj(1)
                emit_pv(1)
                emit_outproj(2)
                emit_outproj(3)
                if debug == 2:
                    for lp in range(2):
                        for r in range(4):
                            nc.sync.dma_start(
                                dbg["dbg_qT_all"]
                                [:, :, 1024 * lp + 256 * r:
                                 1024 * lp + 256 * (r + 1)],
                                qT_all[:, :, lp, r, :])
                    nc.sync.dma_start(dbg["dbg_qn_all"][:], qn_all[:])
                elif debug:
                    nc.sync.dma_start(dbg["dbg_qT_my"][:], qT_my[:])
                    for lp in range(2):
                        for r in range(4):
                            nc.sync.dma_start(
                                dbg["dbg_qT_all"]
                                [:, :, 1024 * lp + 256 * r:
                                 1024 * lp + 256 * (r + 1)],
                                qT_all[:, :, lp, r, :])
                    nc.sync.dma_start(dbg["dbg_qn_all"][:], qn_all[:])
                    off = 0
                    for t in range(4):
                        w = 512 * (t + 1)
                        nc.sync.dma_start(dbg["dbg_E"][:, off:off + w],
                                          E_t[t][:])
                        off += w
                    nc.sync.dma_start(dbg["dbg_hT"][:], hT[:])
                    nc.sync.dma_start(dbg["dbg_hn"][:], hn_sb[:])
                    nc.sync.dma_start(dbg["dbg_weff"][:], weff_bf[:])
                emit_ffn(0)
                emit_ffn(1)

    nc.compile()
    return nc


def _get_nc(debug=False, reps=1, sim_cc_as_dma=False, coresim_safe=False):
    key = ("nc", debug, reps, sim_cc_as_dma, coresim_safe)
    if key not in _CACHE:
        _CACHE[key] = _build(debug, reps, sim_cc_as_dma, coresim_safe)
    return _CACHE[key]


def make_in_maps(x, mask, wi, out_kernel, out_bias):
    """Host-side sharding for the causal fast path (bf16 pre-cast)."""
    import ml_dtypes

    BF = ml_dtypes.bfloat16
    x = np.ascontiguousarray(x, dtype=np.float32)
    wi_b = np.ascontiguousarray(wi, dtype=np.float32).astype(BF)
    ok = np.ascontiguousarray(out_kernel, dtype=np.float32)
    bias_b = np.ascontiguousarray(out_bias, np.float32).reshape(1, D).astype(BF)
    mask = np.asarray(mask).astype(bool)

    in_maps = []
    for c in range(NCORES):
        b, g = divmod(c, 4)
        tiles = _gtiles(g)
        rows = np.concatenate([np.r_[128 * T:128 * (T + 1)] for T in tiles])
        xt = np.ascontiguousarray(x[b, rows, :].T).astype(BF)    # [D, TOK]
        # diagonal-block masks, columns in gather order (rank r, then 128)
        dmask = np.full((4, 128, 512), np.float32(MASK_NEG), np.float32)
        for t in range(4):
            myrows = np.r_[128 * tiles[t]:128 * (tiles[t] + 1)]
            for r in range(4):
                Tc = _gtiles(r)[t]
                cols = np.r_[128 * Tc:128 * (Tc + 1)]
                dmask[t][:, 128 * r:128 * (r + 1)] = np.where(
                    mask[np.ix_(myrows, cols)], np.float32(0.0),
                    np.float32(MASK_NEG))
        wo = np.ascontiguousarray(ok[D * c:D * (c + 1), :]).astype(BF)
        in_maps.append({
            "xt": xt, "wi": wi_b, "wo": wo, "bias": bias_b,
            "dmask": dmask.astype(BF), "ident": np.eye(128, dtype=BF),
        })
    return in_maps


def assemble_output(results):
    out = np.empty((B, S, D), dtype=np.float32)
    for c in range(NCORES):
        b, g = divmod(c, 4)
        res = results[c]["out"]
        for t, T in enumerate(_gtiles(g)):
            out[b, 128 * T:128 * (T + 1), :] = res[128 * t:128 * (t + 1), :]
    return out


def _mask_is_causal(mask):
    mask = np.asarray(mask).astype(bool)
    return mask.shape == (S, S) and np.array_equal(
        mask, np.tril(np.ones((S, S), dtype=bool)))


def kernel(x, mask, wi, out_kernel, out_bias, n_heads):
    from concourse.bass_utils import run_bass_kernel_spmd

    assert int(np.asarray(n_heads)) == H
    if _mask_is_causal(mask):
        nc = _get_nc()
        in_maps = make_in_maps(x, mask, wi, out_kernel, out_bias)
        res = run_bass_kernel_spmd(nc, in_maps, core_ids=list(range(NCORES)))
        return assemble_output(res.results)
    # general-mask fallback (never hit for the reference's causal mask):
    # straightforward numpy evaluation, self-contained and exact.
    x = np.asarray(x, np.float64)
    wi = np.asarray(wi, np.float64)
    out_kernel = np.asarray(out_kernel, np.float64)
    out_bias = np.asarray(out_bias, np.float64)
    mask = np.asarray(mask, bool)
    q = x @ wi
    sc = np.einsum("bqd,bkd->bqk", q, q) / np.sqrt(np.float64(D))
    sc = np.where(mask[None], sc, -1e10)
    sc -= sc.max(axis=-1, keepdims=True)
    p = np.exp(sc)
    p /= p.sum(axis=-1, keepdims=True)
    head = q + p @ q
    h = np.tile(head, (1, 1, H)) @ out_kernel + out_bias
    mean = h.mean(axis=-1, keepdims=True)
    msq = np.square(h).mean(axis=-1, keepdims=True)
    h = (h - mean) / np.sqrt(msq - np.square(mean) + EPS)
    a = h @ wi
    return ((a / (1.0 + np.exp(-a))) @ wi).astype(np.float32)


if __name__ == "__main__":
    # quick self-check against the numpy reference in /tmp/refs.npz
    refs = np.load("/tmp/refs.npz")
    inputs = {k[3:]: refs[k] for k in refs.files if k.startswith("in_")}
    act = kernel(**inputs)
    exp = refs["exp"]
    err = np.linalg.norm(act - exp) / np.linalg.norm(exp)
    print("Relative error:", err)
